# revision 7
# baseline (speedup 1.0000x reference)
"""Trainium2 Bass kernel for nn_Attention_15539191677265.

Single-head-dim attention block:
    qkv = w_qkv @ x ; per-head scaled dot-product attention over w=2048;
    out = w_out @ attn_out + b_out

Sharding: pure data-parallel over batch (b=8 -> 8 NeuronCores, one batch
element per core). Weights are replicated. No collectives.

Per-core algorithm (transposed-softmax, bf16 matmuls):
  1. q,k projections, then DMA-duplicate each head's 64 d-rows into both
     PE row halves so consecutive sim matmuls alternate row groups.
  2. vT = x.T @ wvT with a ones column (AV psum row 64 accumulates the
     softmax normalizer for free).
  3. per (head, i-half 1024, j-tile): sim^T strip as two full-width K=64
     matmuls on alternating PE row-halves (tile_position (0,0)/(64,0)) --
     disjoint rows + different psum banks stream concurrently; exp as ONE
     [128,1024] op, split across engines: most j-tiles on ScalarE (exact
     exp), 6/16 on VectorE via the Schraudolph bf16 bit-trick
     (x*128/ln2 + magic -> int16 bits == bf16 exp approximation);
     AV accumulates [65, 1024] over the 16 j-tiles.
  4. normalize: reciprocal of the ones-row (DVE), broadcast (GpSimd),
     multiply (GpSimd), odd heads bounce partitions via DMA.
  5. proj: out = woutT.T @ attn_out (K=128 head-pair chunks) + bias.

No max subtraction before exp: scores are ~N(0,1) so exp cannot
overflow in bf16/fp32.
"""

import sys

if "/opt/trn_rl_repo" not in sys.path:
    sys.path.insert(0, "/opt/trn_rl_repo")

import numpy as np
import ml_dtypes

import concourse.bass as bass
import concourse.mybir as mybir
import concourse.tile as tile
from concourse import bacc
from concourse.bass_utils import run_bass_kernel_spmd

BF16 = mybir.dt.bfloat16
F32 = mybir.dt.float32
I16 = mybir.dt.int16
EXP = mybir.ActivationFunctionType.Exp

B, DIM, W = 8, 256, 2048
HEADS, DH = 8, 64
HID = HEADS * DH  # 512
SCALE = DH ** (-0.5)
N_CORES = 8

NJT = W // 128  # 16 j-tiles per head
NCT = DIM // 128  # 2 contraction chunks over channels
IH = 1024  # i-half width
NIH = W // IH

# j-tiles whose exp runs on VectorE via Schraudolph (rest: exact on ScalarE)
DVE_JTS = frozenset({1, 4, 6, 9, 11, 14})
SCH_MUL = 128.0 / float(np.log(2.0))  # 184.6645
SCH_ADD = 127.0 * 128.0 - 7.4  # PWL-centering constant (round-to-nearest)


def build_kernel():
    nc = bacc.Bacc(None, target_bir_lowering=False)

    x_d = nc.dram_tensor("x", [DIM, W], BF16, kind="ExternalInput")
    wqkvT_d = nc.dram_tensor("wqkvT", [DIM, 3 * HID], BF16, kind="ExternalInput")
    woutT_d = nc.dram_tensor("woutT", [128, 4, DIM], BF16, kind="ExternalInput")
    bias_d = nc.dram_tensor("bias", [128, DIM // 128], F32, kind="ExternalInput")
    out_d = nc.dram_tensor("out", [DIM, W], F32, kind="ExternalOutput")

    with tile.TileContext(nc) as tc:
        with tc.tile_pool(name="pers", bufs=1) as pers:
            x_sb = pers.tile([128, NCT, W], BF16)
            wq_sb = pers.tile([128, NCT, 3 * HID], BF16)
            wo_sb = pers.tile([128, 4, DIM], BF16)
            bias_sb = pers.tile([128, DIM // 128], F32)
            # [128 = head d-rows duplicated in both halves, head, w]
            q_sb = pers.tile([128, HEADS, W], BF16)
            k_sb = pers.tile([128, HEADS, W], BF16)
            vt_sb = pers.tile([128, NJT, HEADS, 128], BF16)
            attout_sb = [
                pers.tile([128, W], BF16, name=f"attout_{kc}", tag=f"attout{kc}")
                for kc in range(4)
            ]
            out_sb = pers.tile([128, NCT, W], F32)

            xr = x_d[:].rearrange("(ct p) w -> p ct w", p=128)
            for ct in range(NCT):
                for wh in range(4):
                    nc.sync.dma_start(
                        out=x_sb[:, ct, wh * 512 : (wh + 1) * 512],
                        in_=xr[:, ct, wh * 512 : (wh + 1) * 512],
                    )
            wqr = wqkvT_d[:].rearrange("(ct p) o -> p ct o", p=128)
            for ct in range(NCT):
                for sec in range(3):
                    nc.sync.dma_start(
                        out=wq_sb[:, ct, sec * HID : (sec + 1) * HID],
                        in_=wqr[:, ct, sec * HID : (sec + 1) * HID],
                    )
            nc.sync.dma_start(out=wo_sb[:], in_=woutT_d[:])
            nc.sync.dma_start(out=bias_sb[:], in_=bias_d[:])

            # vT cols 64..127: ones column then zero padding
            nc.vector.memset(vt_sb[:, :, :, DH:128], 0.0)
            nc.vector.memset(vt_sb[:, :, :, DH : DH + 1], 1.0)
            # warm the ACT exp table while qkv matmuls run
            warm = pers.tile([1, 1], F32)
            nc.vector.memset(warm[:], 0.0)
            nc.scalar.activation(out=warm[:], in_=warm[:], func=EXP)

            # ---- phase 1: v first (attention h0 needs all of vT), then q,k
            with tc.tile_pool(name="qkv_ps", bufs=4, space="PSUM") as qkv_ps:
                for jt in range(NJT):
                    ps = qkv_ps.tile([128, HID], F32, name=f"vt_{jt}", tag="vt")
                    for ct in range(NCT):
                        nc.tensor.matmul(
                            ps[:],
                            lhsT=x_sb[:, ct, jt * 128 : (jt + 1) * 128],
                            rhs=wq_sb[:, ct, 2 * HID : 3 * HID],
                            start=(ct == 0),
                            stop=(ct == NCT - 1),
                        )
                    if jt % 2 == 0:
                        nc.vector.tensor_copy(
                            out=vt_sb[:, jt, :, 0:DH],
                            in_=ps[:].rearrange("p (h d) -> p h d", h=HEADS),
                        )
                    else:
                        nc.scalar.copy(
                            out=vt_sb[:, jt, :, 0:DH],
                            in_=ps[:].rearrange("p (h d) -> p h d", h=HEADS),
                        )

                # q,k per o-tile (2 heads each); evac halves to their head
                # slots, then DMA duplicates into the other partition half
                for ot in range(4):
                    for dst, base in ((q_sb, 0), (k_sb, HID)):
                        for ph in range(4):
                            po = ph * 512
                            ps = qkv_ps.tile(
                                [128, 512], F32, name=f"qk_{ot}_{base}_{ph}", tag="qk"
                            )
                            for ct in range(NCT):
                                nc.tensor.matmul(
                                    ps[:],
                                    lhsT=wq_sb[:, ct, base + ot * 128 : base + (ot + 1) * 128],
                                    rhs=x_sb[:, ct, po : po + 512],
                                    start=(ct == 0),
                                    stop=(ct == NCT - 1),
                                )
                            if ph % 2 == 0:
                                nc.vector.tensor_copy(
                                    out=dst[0:64, 2 * ot, po : po + 512],
                                    in_=ps[0:64, :],
                                )
                                nc.scalar.copy(
                                    out=dst[64:128, 2 * ot + 1, po : po + 512],
                                    in_=ps[64:128, :],
                                )
                            else:
                                nc.scalar.copy(
                                    out=dst[0:64, 2 * ot, po : po + 512],
                                    in_=ps[0:64, :],
                                )
                                nc.vector.tensor_copy(
                                    out=dst[64:128, 2 * ot + 1, po : po + 512],
                                    in_=ps[64:128, :],
                                )
                        # partition-duplicating DMAs (overlap with matmuls)
                        nc.sync.dma_start(
                            out=dst[64:128, 2 * ot, :], in_=dst[0:64, 2 * ot, :]
                        )
                        nc.sync.dma_start(
                            out=dst[0:64, 2 * ot + 1, :], in_=dst[64:128, 2 * ot + 1, :]
                        )

            # ---- phase 3: attention ----
            with (
                tc.tile_pool(name="strip_ps", bufs=2, space="PSUM") as strip_ps,
                tc.tile_pool(name="av_ps", bufs=2, space="PSUM") as av_ps,
                tc.tile_pool(name="exp_sb", bufs=4) as exp_pool,
                tc.tile_pool(name="norm_sb", bufs=2) as norm_pool,
            ):
                for h in range(HEADS):
                    for ih in range(NIH):
                        io = ih * IH
                        av = av_ps.tile([128, IH], F32, name=f"av_{h}_{ih}", tag="av")
                        for jt in range(NJT):
                            strip = strip_ps.tile(
                                [128, IH], F32, name=f"st_{h}_{ih}_{jt}", tag="st"
                            )
                            for c in range(IH // 512):
                                co = c * 512
                                # full-width K=64 matmuls on alternating PE
                                # row-halves: adjacent ops use disjoint rows
                                # and different psum banks -> concurrent
                                rg = 64 * (c % 2)
                                nc.tensor.matmul(
                                    strip[:, co : co + 512],
                                    lhsT=k_sb[rg : rg + 64, h, jt * 128 : (jt + 1) * 128],
                                    rhs=q_sb[rg : rg + 64, h, io + co : io + co + 512],
                                    start=True,
                                    stop=True,
                                    tile_position=(rg, 0),
                                )
                            es = exp_pool.tile([128, IH], BF16, name=f"es_{jt}", tag="es")
                            if jt in DVE_JTS:
                                nc.vector.tensor_scalar(
                                    out=es[:].bitcast(I16),
                                    in0=strip[:],
                                    scalar1=SCH_MUL,
                                    scalar2=SCH_ADD,
                                    op0=mybir.AluOpType.mult,
                                    op1=mybir.AluOpType.add,
                                )
                            else:
                                nc.scalar.activation(out=es[:], in_=strip[:], func=EXP)
                            for c in range(IH // 512):
                                co = c * 512
                                nc.tensor.matmul(
                                    av[0 : DH + 1, co : co + 512],
                                    lhsT=vt_sb[:, jt, h, 0 : DH + 1],
                                    rhs=es[:, co : co + 512],
                                    start=(jt == 0),
                                    stop=(jt == NJT - 1),
                                )
                        # normalize off the critical path
                        avc = norm_pool.tile([DH + 1, IH], F32, tag="avc")
                        if h % 2 == 0:
                            nc.vector.tensor_copy(out=avc[:], in_=av[0 : DH + 1, :])
                        else:
                            nc.scalar.copy(out=avc[:], in_=av[0 : DH + 1, :])
                        rec0 = norm_pool.tile([1, IH], F32, tag="rec0")
                        bcn = norm_pool.tile([DH, IH], F32, tag="bcn")
                        bc = norm_pool.tile([DH, IH], F32, tag="bc")
                        nc.sync.dma_start(out=rec0[:], in_=avc[DH : DH + 1, :])
                        nc.gpsimd.partition_broadcast(bcn[:], rec0[0:1, :], channels=DH)
                        nc.vector.reciprocal_approx_fast(out=bc[:], in_=bcn[:])
                        if h % 2 == 0:
                            nc.gpsimd.tensor_mul(
                                out=attout_sb[h // 2][0:DH, io : io + IH],
                                in0=avc[0:DH, :],
                                in1=bc[:],
                            )
                        else:
                            # odd heads land on partitions 64..127: bounce
                            atmp = norm_pool.tile([DH, IH], BF16, tag="atmp")
                            nc.gpsimd.tensor_mul(out=atmp[:], in0=avc[0:DH, :], in1=bc[:])
                            nc.sync.dma_start(
                                out=attout_sb[h // 2][DH:128, io : io + IH],
                                in_=atmp[:],
                            )

            # ---- phase 4: output projection + bias ----
            outr = out_d[:].rearrange("(ct p) w -> p ct w", p=128)
            with tc.tile_pool(name="proj_ps", bufs=8, space="PSUM") as proj_ps:
                for ot in range(NCT):
                    for wh in range(4):
                        wo = wh * 512
                        ps = proj_ps.tile([128, 512], F32, name=f"pj_{ot}_{wh}", tag="pj")
                        for kc in range(4):
                            nc.tensor.matmul(
                                ps[:],
                                lhsT=wo_sb[:, kc, ot * 128 : (ot + 1) * 128],
                                rhs=attout_sb[kc][:, wo : wo + 512],
                                start=(kc == 0),
                                stop=(kc == 3),
                            )
                        nc.vector.tensor_scalar_add(
                            out=out_sb[:, ot, wo : wo + 512],
                            in0=ps[:],
                            scalar1=bias_sb[:, ot : ot + 1],
                        )
                        nc.sync.dma_start(
                            out=outr[:, ot, wo : wo + 512],
                            in_=out_sb[:, ot, wo : wo + 512],
                        )

    nc.compile()
    return nc


_NC_CACHE = None


def _get_nc():
    global _NC_CACHE
    if _NC_CACHE is None:
        _NC_CACHE = build_kernel()
    return _NC_CACHE


def make_in_maps(x, w_qkv, w_out, b_out):
    bf16 = ml_dtypes.bfloat16
    wq = np.array(w_qkv, dtype=np.float32, copy=True)
    wq[:HID] *= SCALE  # fold attention scale into the q projection
    wqkvT = np.ascontiguousarray(wq.T).astype(bf16)  # [256, 1536]
    woutT = np.ascontiguousarray(
        w_out.T.reshape(4, 128, DIM).transpose(1, 0, 2)
    ).astype(bf16)  # [128, 4, 256]
    bias = np.ascontiguousarray(
        b_out.astype(np.float32).reshape(DIM // 128, 128).T
    )  # [128, 2]
    in_maps = []
    for i in range(N_CORES):
        in_maps.append(
            {
                "x": x[i].astype(bf16),
                "wqkvT": wqkvT,
                "woutT": woutT,
                "bias": bias,
            }
        )
    return in_maps


def kernel(x, w_qkv, w_out, b_out, _trace=False):
    nc = _get_nc()
    in_maps = make_in_maps(x, w_qkv, w_out, b_out)
    res = run_bass_kernel_spmd(
        nc,
        in_maps,
        core_ids=list(range(N_CORES)),
        trace=_trace,
        trace_cores=list(range(N_CORES)) if _trace else None,
    )
    out = np.stack([res.results[i]["out"] for i in range(N_CORES)], axis=0)
    if _trace:
        kernel.last_exec_time_ns = res.exec_time_ns
        kernel.last_results = res
    return out


# revision 13
# speedup vs baseline: 1.1005x; 1.1005x over previous
"""Trainium2 Bass kernel for nn_Attention_15539191677265.

Single-head-dim attention block:
    qkv = w_qkv @ x ; per-head scaled dot-product attention over w=2048;
    out = w_out @ attn_out + b_out

Sharding: pure data-parallel over batch (b=8 -> 8 NeuronCores, one batch
element per core). Weights are replicated. No collectives.

Per-core algorithm (transposed-softmax, bf16 matmuls):
  1. q,k projections, then DMA-duplicate each head's 64 d-rows into both
     PE row halves so consecutive sim matmuls alternate row groups.
  2. vT = x.T @ wvT with a ones column (AV psum row 64 accumulates the
     softmax normalizer for free).
  3. per (head, i-half 1024, j-tile): sim^T strip as two full-width K=64
     matmuls on alternating PE row-halves (tile_position (0,0)/(64,0)) --
     disjoint rows + different psum banks stream concurrently; exp as ONE
     [128,1024] op, split across engines: most j-tiles on ScalarE (exact
     exp), 6/16 on VectorE via the Schraudolph bf16 bit-trick
     (x*128/ln2 + magic -> int16 bits == bf16 exp approximation);
     AV accumulates [65, 1024] over the 16 j-tiles.
  4. normalize: reciprocal of the ones-row (DVE), broadcast (GpSimd),
     multiply (GpSimd), odd heads bounce partitions via DMA.
  5. proj: out = woutT.T @ attn_out (K=128 head-pair chunks) + bias.

No max subtraction before exp: scores are ~N(0,1) so exp cannot
overflow in bf16/fp32.
"""

import sys

if "/opt/trn_rl_repo" not in sys.path:
    sys.path.insert(0, "/opt/trn_rl_repo")

import numpy as np
import ml_dtypes

import concourse.bass as bass
import concourse.mybir as mybir
import concourse.tile as tile
from concourse import bacc
from concourse.bass_utils import run_bass_kernel_spmd

BF16 = mybir.dt.bfloat16
F32 = mybir.dt.float32
I16 = mybir.dt.int16
EXP = mybir.ActivationFunctionType.Exp

B, DIM, W = 8, 256, 2048
HEADS, DH = 8, 64
HID = HEADS * DH  # 512
SCALE = DH ** (-0.5)
N_CORES = 8

NJT = W // 128  # 16 j-tiles per head
NCT = DIM // 128  # 2 contraction chunks over channels
IH = 1024  # i-half width
NIH = W // IH

# j-tiles whose exp runs on VectorE via Schraudolph (rest: exact on ScalarE)
DVE_JTS = frozenset({1, 3, 5, 8, 10, 12, 14})
SCH_MUL = 128.0 / float(np.log(2.0))  # 184.6645
SCH_ADD = 127.0 * 128.0 - 7.4  # PWL-centering constant (round-to-nearest)


def build_kernel():
    nc = bacc.Bacc(None, target_bir_lowering=False)

    x_d = nc.dram_tensor("x", [DIM, W], BF16, kind="ExternalInput")
    wqkvT_d = nc.dram_tensor("wqkvT", [DIM, 3 * HID], BF16, kind="ExternalInput")
    woutT_d = nc.dram_tensor("woutT", [128, 4, DIM], BF16, kind="ExternalInput")
    bias_d = nc.dram_tensor("bias", [128, DIM // 128], F32, kind="ExternalInput")
    out_d = nc.dram_tensor("out", [DIM, W], F32, kind="ExternalOutput")

    with tile.TileContext(nc) as tc:
        with tc.tile_pool(name="pers", bufs=1) as pers:
            x_sb = pers.tile([128, NCT, W], BF16)
            wq_sb = pers.tile([128, NCT, 3 * HID], BF16)
            wo_sb = pers.tile([128, 4, DIM], BF16)
            bias_sb = pers.tile([128, DIM // 128], F32)
            # [128 = head d-rows duplicated in both halves, head, w]
            q_sb = pers.tile([128, HEADS, W], BF16)
            k_sb = pers.tile([128, HEADS, W], BF16)
            vt_sb = pers.tile([128, NJT, HEADS, 128], BF16)
            attout_sb = [
                pers.tile([128, W], BF16, name=f"attout_{kc}", tag=f"attout{kc}")
                for kc in range(4)
            ]
            out_sb = pers.tile([128, NCT, W], F32)

            xr = x_d[:].rearrange("(ct p) w -> p ct w", p=128)
            for ct in range(NCT):
                for wh in range(4):
                    nc.sync.dma_start(
                        out=x_sb[:, ct, wh * 512 : (wh + 1) * 512],
                        in_=xr[:, ct, wh * 512 : (wh + 1) * 512],
                    )
            wqr = wqkvT_d[:].rearrange("(ct p) o -> p ct o", p=128)
            for ct in range(NCT):
                for sec in range(3):
                    nc.sync.dma_start(
                        out=wq_sb[:, ct, sec * HID : (sec + 1) * HID],
                        in_=wqr[:, ct, sec * HID : (sec + 1) * HID],
                    )
            nc.sync.dma_start(out=wo_sb[:], in_=woutT_d[:])
            nc.sync.dma_start(out=bias_sb[:], in_=bias_d[:])

            # vT cols 64..127: ALL ones -> AV rows 64..127 hold the softmax
            # normalizer replicated across 64 partitions (pre-broadcast)
            nc.vector.memset(vt_sb[:, :, :, DH:128], 1.0)
            # warm the ACT exp table while qkv matmuls run
            warm = pers.tile([1, 1], F32)
            nc.vector.memset(warm[:], 0.0)
            nc.scalar.activation(out=warm[:], in_=warm[:], func=EXP)

            # ---- phase 1: v first (attention h0 needs all of vT), then q,k
            with tc.tile_pool(name="qkv_ps", bufs=4, space="PSUM") as qkv_ps:
                for jt in range(NJT):
                    ps = qkv_ps.tile([128, HID], F32, name=f"vt_{jt}", tag="vt")
                    for ct in range(NCT):
                        nc.tensor.matmul(
                            ps[:],
                            lhsT=x_sb[:, ct, jt * 128 : (jt + 1) * 128],
                            rhs=wq_sb[:, ct, 2 * HID : 3 * HID],
                            start=(ct == 0),
                            stop=(ct == NCT - 1),
                        )
                    if jt % 2 == 0:
                        nc.vector.tensor_copy(
                            out=vt_sb[:, jt, :, 0:DH],
                            in_=ps[:].rearrange("p (h d) -> p h d", h=HEADS),
                        )
                    else:
                        nc.scalar.copy(
                            out=vt_sb[:, jt, :, 0:DH],
                            in_=ps[:].rearrange("p (h d) -> p h d", h=HEADS),
                        )

                # q,k per o-tile (2 heads each); evac halves to their head
                # slots, then DMA duplicates into the other partition half
                for ot in range(4):
                    for dst, base in ((q_sb, 0), (k_sb, HID)):
                        for ph in range(4):
                            po = ph * 512
                            ps = qkv_ps.tile(
                                [128, 512], F32, name=f"qk_{ot}_{base}_{ph}", tag="qk"
                            )
                            for ct in range(NCT):
                                nc.tensor.matmul(
                                    ps[:],
                                    lhsT=wq_sb[:, ct, base + ot * 128 : base + (ot + 1) * 128],
                                    rhs=x_sb[:, ct, po : po + 512],
                                    start=(ct == 0),
                                    stop=(ct == NCT - 1),
                                )
                            if ph % 2 == 0:
                                nc.vector.tensor_copy(
                                    out=dst[0:64, 2 * ot, po : po + 512],
                                    in_=ps[0:64, :],
                                )
                                nc.scalar.copy(
                                    out=dst[64:128, 2 * ot + 1, po : po + 512],
                                    in_=ps[64:128, :],
                                )
                            else:
                                nc.scalar.copy(
                                    out=dst[0:64, 2 * ot, po : po + 512],
                                    in_=ps[0:64, :],
                                )
                                nc.vector.tensor_copy(
                                    out=dst[64:128, 2 * ot + 1, po : po + 512],
                                    in_=ps[64:128, :],
                                )
                        # partition-duplicating DMAs (overlap with matmuls)
                        nc.sync.dma_start(
                            out=dst[64:128, 2 * ot, :], in_=dst[0:64, 2 * ot, :]
                        )
                        nc.sync.dma_start(
                            out=dst[0:64, 2 * ot + 1, :], in_=dst[64:128, 2 * ot + 1, :]
                        )

            # ---- phase 3: attention ----
            with (
                tc.tile_pool(name="strip_ps", bufs=2, space="PSUM") as strip_ps,
                tc.tile_pool(name="av_ps", bufs=2, space="PSUM") as av_ps,
                tc.tile_pool(name="exp_sb", bufs=4) as exp_pool,
                tc.tile_pool(name="norm_sb", bufs=2) as norm_pool,
            ):
                def emit_av(av, h, jt):
                    es = es_tiles[jt]
                    for c in range(IH // 512):
                        co = c * 512
                        nc.tensor.matmul(
                            av[:, co : co + 512],
                            lhsT=vt_sb[:, jt, h, :],
                            rhs=es[:, co : co + 512],
                            start=(jt == 0),
                            stop=(jt == NJT - 1),
                        )

                for h in range(HEADS):
                    for ih in range(NIH):
                        io = ih * IH
                        av = av_ps.tile([128, IH], F32, name=f"av_{h}_{ih}", tag="av")
                        es_tiles = {}
                        for jt in range(NJT):
                            strip = strip_ps.tile(
                                [128, IH], F32, name=f"st_{h}_{ih}_{jt}", tag="st"
                            )
                            for c in range(IH // 512):
                                co = c * 512
                                # full-width K=64 matmuls on alternating PE
                                # row-halves: adjacent ops use disjoint rows
                                # and different psum banks -> concurrent
                                rg = 64 * (c % 2)
                                nc.tensor.matmul(
                                    strip[:, co : co + 512],
                                    lhsT=k_sb[rg : rg + 64, h, jt * 128 : (jt + 1) * 128],
                                    rhs=q_sb[rg : rg + 64, h, io + co : io + co + 512],
                                    start=True,
                                    stop=True,
                                    tile_position=(rg, 0),
                                )
                            es = exp_pool.tile([128, IH], BF16, name=f"es_{jt}", tag="es")
                            es_tiles[jt] = es
                            if jt in DVE_JTS:
                                nc.vector.tensor_scalar(
                                    out=es[:].bitcast(I16),
                                    in0=strip[:],
                                    scalar1=SCH_MUL,
                                    scalar2=SCH_ADD,
                                    op0=mybir.AluOpType.mult,
                                    op1=mybir.AluOpType.add,
                                )
                            else:
                                nc.scalar.activation(out=es[:], in_=strip[:], func=EXP)
                            # software-pipelined: AV lags sim by one j-tile so
                            # the PE never parks behind the exp engines
                            if jt > 0:
                                emit_av(av, h, jt - 1)
                        emit_av(av, h, NJT - 1)
                        # normalize: rows 64..127 of av already hold the
                        # normalizer replicated; shift it down via DMA, then
                        # one DVE divide finishes the softmax
                        ncp = norm_pool.tile([128, IH], F32, tag="ncp")
                        if h % 2 == 0:
                            nc.scalar.copy(out=ncp[DH:128, :], in_=av[DH:128, :])
                        else:
                            nc.vector.tensor_copy(out=ncp[DH:128, :], in_=av[DH:128, :])
                        nsh = norm_pool.tile([DH, IH], F32, tag="nsh")
                        nc.sync.dma_start(out=nsh[:], in_=ncp[DH:128, :])
                        # custom-DVE ops require base partition 0
                        nrm = norm_pool.tile([DH, IH], F32, tag="nrm")
                        nc.vector.reciprocal_approx_fast(out=nrm[:], in_=nsh[:])
                        if h % 2 == 0:
                            nc.vector.tensor_mul(
                                out=attout_sb[h // 2][0:DH, io : io + IH],
                                in0=av[0:DH, :],
                                in1=nrm[:],
                            )
                        else:
                            # odd heads land on partitions 64..127: bounce
                            atmp = norm_pool.tile([DH, IH], BF16, tag="atmp")
                            nc.vector.tensor_mul(
                                out=atmp[:], in0=av[0:DH, :], in1=nrm[:]
                            )
                            nc.sync.dma_start(
                                out=attout_sb[h // 2][DH:128, io : io + IH],
                                in_=atmp[:],
                            )

            # ---- phase 4: output projection + bias ----
            outr = out_d[:].rearrange("(ct p) w -> p ct w", p=128)
            with tc.tile_pool(name="proj_ps", bufs=8, space="PSUM") as proj_ps:
                for ot in range(NCT):
                    for wh in range(4):
                        wo = wh * 512
                        ps = proj_ps.tile([128, 512], F32, name=f"pj_{ot}_{wh}", tag="pj")
                        for kc in range(4):
                            nc.tensor.matmul(
                                ps[:],
                                lhsT=wo_sb[:, kc, ot * 128 : (ot + 1) * 128],
                                rhs=attout_sb[kc][:, wo : wo + 512],
                                start=(kc == 0),
                                stop=(kc == 3),
                            )
                        nc.vector.tensor_scalar_add(
                            out=out_sb[:, ot, wo : wo + 512],
                            in0=ps[:],
                            scalar1=bias_sb[:, ot : ot + 1],
                        )
                        nc.sync.dma_start(
                            out=outr[:, ot, wo : wo + 512],
                            in_=out_sb[:, ot, wo : wo + 512],
                        )

    nc.compile()
    return nc


_NC_CACHE = None


def _get_nc():
    global _NC_CACHE
    if _NC_CACHE is None:
        _NC_CACHE = build_kernel()
    return _NC_CACHE


def make_in_maps(x, w_qkv, w_out, b_out):
    bf16 = ml_dtypes.bfloat16
    wq = np.array(w_qkv, dtype=np.float32, copy=True)
    wq[:HID] *= SCALE  # fold attention scale into the q projection
    wqkvT = np.ascontiguousarray(wq.T).astype(bf16)  # [256, 1536]
    woutT = np.ascontiguousarray(
        w_out.T.reshape(4, 128, DIM).transpose(1, 0, 2)
    ).astype(bf16)  # [128, 4, 256]
    bias = np.ascontiguousarray(
        b_out.astype(np.float32).reshape(DIM // 128, 128).T
    )  # [128, 2]
    in_maps = []
    for i in range(N_CORES):
        in_maps.append(
            {
                "x": x[i].astype(bf16),
                "wqkvT": wqkvT,
                "woutT": woutT,
                "bias": bias,
            }
        )
    return in_maps


def kernel(x, w_qkv, w_out, b_out, _trace=False):
    nc = _get_nc()
    in_maps = make_in_maps(x, w_qkv, w_out, b_out)
    res = run_bass_kernel_spmd(
        nc,
        in_maps,
        core_ids=list(range(N_CORES)),
        trace=_trace,
        trace_cores=list(range(N_CORES)) if _trace else None,
    )
    out = np.stack([res.results[i]["out"] for i in range(N_CORES)], axis=0)
    if _trace:
        kernel.last_exec_time_ns = res.exec_time_ns
        kernel.last_results = res
    return out


# revision 14
# speedup vs baseline: 1.4878x; 1.3519x over previous
"""Trainium2 Bass kernel for nn_Attention_15539191677265.

Single-head-dim attention block:
    qkv = w_qkv @ x ; per-head scaled dot-product attention over w=2048;
    out = w_out @ attn_out + b_out

Sharding: pure data-parallel over batch (b=8 -> 8 NeuronCores, one batch
element per core). Weights are replicated. No collectives.

Per-core algorithm (transposed-softmax, bf16 matmuls):
  1. q,k projections, then DMA-duplicate each head's 64 d-rows into both
     PE row halves so consecutive sim matmuls alternate row groups.
  2. vT = x.T @ wvT with a ones column (AV psum row 64 accumulates the
     softmax normalizer for free).
  3. per (head, i-half 1024, j-tile): sim^T strip as two full-width K=64
     matmuls on alternating PE row-halves (tile_position (0,0)/(64,0)) --
     disjoint rows + different psum banks stream concurrently; exp as ONE
     [128,1024] op, split across engines: most j-tiles on ScalarE (exact
     exp), 6/16 on VectorE via the Schraudolph bf16 bit-trick
     (x*128/ln2 + magic -> int16 bits == bf16 exp approximation);
     AV accumulates [65, 1024] over the 16 j-tiles.
  4. normalize: reciprocal of the ones-row (DVE), broadcast (GpSimd),
     multiply (GpSimd), odd heads bounce partitions via DMA.
  5. proj: out = woutT.T @ attn_out (K=128 head-pair chunks) + bias.

No max subtraction before exp: scores are ~N(0,1) so exp cannot
overflow in bf16/fp32.
"""

import sys

if "/opt/trn_rl_repo" not in sys.path:
    sys.path.insert(0, "/opt/trn_rl_repo")

import numpy as np
import ml_dtypes

import concourse.bass as bass
import concourse.mybir as mybir
import concourse.tile as tile
from concourse import bacc
from concourse.bass_utils import run_bass_kernel_spmd

BF16 = mybir.dt.bfloat16
F32 = mybir.dt.float32
I16 = mybir.dt.int16
EXP = mybir.ActivationFunctionType.Exp

B, DIM, W = 8, 256, 2048
HEADS, DH = 8, 64
HID = HEADS * DH  # 512
SCALE = DH ** (-0.5)
N_CORES = 8

NJT = W // 128  # 16 j-tiles per head
NCT = DIM // 128  # 2 contraction chunks over channels
IH = 1024  # i-half width
NIH = W // IH

# j-tiles whose exp runs on VectorE via Schraudolph (rest: exact on ScalarE)
DVE_JTS = frozenset({1, 3, 5, 8, 10, 12, 14})
SCH_MUL = 128.0 / float(np.log(2.0))  # 184.6645
SCH_ADD = 127.0 * 128.0 - 7.4  # PWL-centering constant (round-to-nearest)


def build_kernel():
    nc = bacc.Bacc(None, target_bir_lowering=False)

    x_d = nc.dram_tensor("x", [DIM, W], BF16, kind="ExternalInput")
    wqkvT_d = nc.dram_tensor("wqkvT", [DIM, 3 * HID], BF16, kind="ExternalInput")
    woutT_d = nc.dram_tensor("woutT", [128, 4, DIM], BF16, kind="ExternalInput")
    bias_d = nc.dram_tensor("bias", [128, DIM // 128], F32, kind="ExternalInput")
    out_d = nc.dram_tensor("out", [DIM, W], F32, kind="ExternalOutput")

    with tile.TileContext(nc) as tc:
        with tc.tile_pool(name="pers", bufs=1) as pers:
            x_sb = pers.tile([128, NCT, W], BF16)
            wq_sb = pers.tile([128, NCT, 3 * HID], BF16)
            wo_sb = pers.tile([128, 4, DIM], BF16)
            bias_sb = pers.tile([128, DIM // 128], F32)
            # [128 = head d-rows duplicated in both halves, head, w]
            q_sb = pers.tile([128, HEADS, W], BF16)
            k_sb = pers.tile([128, HEADS, W], BF16)
            vt_sb = pers.tile([128, NJT, HEADS, 128], BF16)
            attout_sb = [
                pers.tile([128, W], BF16, name=f"attout_{kc}", tag=f"attout{kc}")
                for kc in range(4)
            ]
            out_sb = pers.tile([128, NCT, W], F32)

            xr = x_d[:].rearrange("(ct p) w -> p ct w", p=128)
            for ct in range(NCT):
                for wh in range(4):
                    nc.sync.dma_start(
                        out=x_sb[:, ct, wh * 512 : (wh + 1) * 512],
                        in_=xr[:, ct, wh * 512 : (wh + 1) * 512],
                    )
            wqr = wqkvT_d[:].rearrange("(ct p) o -> p ct o", p=128)
            for ct in range(NCT):
                for sec in range(3):
                    nc.sync.dma_start(
                        out=wq_sb[:, ct, sec * HID : (sec + 1) * HID],
                        in_=wqr[:, ct, sec * HID : (sec + 1) * HID],
                    )
            nc.sync.dma_start(out=wo_sb[:], in_=woutT_d[:])
            nc.sync.dma_start(out=bias_sb[:], in_=bias_d[:])

            # vT cols 64..127: ALL ones -> AV rows 64..127 hold the softmax
            # normalizer replicated across 64 partitions (pre-broadcast)
            nc.vector.memset(vt_sb[:, :, :, DH:128], 1.0)
            # warm the ACT exp table while qkv matmuls run
            warm = pers.tile([1, 1], F32)
            nc.vector.memset(warm[:], 0.0)
            nc.scalar.activation(out=warm[:], in_=warm[:], func=EXP)

            # ---- phase 1: v first (attention h0 needs all of vT), then q,k
            with tc.tile_pool(name="qkv_ps", bufs=4, space="PSUM") as qkv_ps:
                for jt in range(NJT):
                    ps = qkv_ps.tile([128, HID], F32, name=f"vt_{jt}", tag="vt")
                    for ct in range(NCT):
                        nc.tensor.matmul(
                            ps[:],
                            lhsT=x_sb[:, ct, jt * 128 : (jt + 1) * 128],
                            rhs=wq_sb[:, ct, 2 * HID : 3 * HID],
                            start=(ct == 0),
                            stop=(ct == NCT - 1),
                        )
                    if jt % 2 == 0:
                        nc.vector.tensor_copy(
                            out=vt_sb[:, jt, :, 0:DH],
                            in_=ps[:].rearrange("p (h d) -> p h d", h=HEADS),
                        )
                    else:
                        nc.scalar.copy(
                            out=vt_sb[:, jt, :, 0:DH],
                            in_=ps[:].rearrange("p (h d) -> p h d", h=HEADS),
                        )

                # q,k per o-tile (2 heads each); evac halves to their head
                # slots, then DMA duplicates into the other partition half
                for ot in range(4):
                    for dst, base in ((q_sb, 0), (k_sb, HID)):
                        for ph in range(4):
                            po = ph * 512
                            ps = qkv_ps.tile(
                                [128, 512], F32, name=f"qk_{ot}_{base}_{ph}", tag="qk"
                            )
                            for ct in range(NCT):
                                nc.tensor.matmul(
                                    ps[:],
                                    lhsT=wq_sb[:, ct, base + ot * 128 : base + (ot + 1) * 128],
                                    rhs=x_sb[:, ct, po : po + 512],
                                    start=(ct == 0),
                                    stop=(ct == NCT - 1),
                                )
                            if ph % 2 == 0:
                                nc.vector.tensor_copy(
                                    out=dst[0:64, 2 * ot, po : po + 512],
                                    in_=ps[0:64, :],
                                )
                                nc.scalar.copy(
                                    out=dst[64:128, 2 * ot + 1, po : po + 512],
                                    in_=ps[64:128, :],
                                )
                            else:
                                nc.scalar.copy(
                                    out=dst[0:64, 2 * ot, po : po + 512],
                                    in_=ps[0:64, :],
                                )
                                nc.vector.tensor_copy(
                                    out=dst[64:128, 2 * ot + 1, po : po + 512],
                                    in_=ps[64:128, :],
                                )
                        # partition-duplicating DMAs (overlap with matmuls)
                        nc.sync.dma_start(
                            out=dst[64:128, 2 * ot, :], in_=dst[0:64, 2 * ot, :]
                        )
                        nc.sync.dma_start(
                            out=dst[0:64, 2 * ot + 1, :], in_=dst[64:128, 2 * ot + 1, :]
                        )

            # ---- phase 3: attention ----
            with (
                tc.tile_pool(name="strip_ps", bufs=2, space="PSUM") as strip_ps,
                tc.tile_pool(name="av_ps", bufs=2, space="PSUM") as av_ps,
                tc.tile_pool(name="exp_sb", bufs=4) as exp_pool,
                tc.tile_pool(name="norm_sb", bufs=2) as norm_pool,
            ):
                def emit_av(av, h, jt):
                    es = es_tiles[jt]
                    for c in range(IH // 512):
                        co = c * 512
                        nc.tensor.matmul(
                            av[:, co : co + 512],
                            lhsT=vt_sb[:, jt, h, :],
                            rhs=es[:, co : co + 512],
                            start=(jt == 0),
                            stop=(jt == NJT - 1),
                        )

                for h in range(HEADS):
                    for ih in range(NIH):
                        io = ih * IH
                        av = av_ps.tile([128, IH], F32, name=f"av_{h}_{ih}", tag="av")
                        es_tiles = {}
                        for jt in range(NJT):
                            strip = strip_ps.tile(
                                [128, IH], F32, name=f"st_{h}_{ih}_{jt}", tag="st"
                            )
                            for c in range(IH // 512):
                                co = c * 512
                                # full-width K=64 matmuls on alternating PE
                                # row-halves: adjacent ops use disjoint rows
                                # and different psum banks -> concurrent
                                rg = 64 * (c % 2)
                                nc.tensor.matmul(
                                    strip[:, co : co + 512],
                                    lhsT=k_sb[rg : rg + 64, h, jt * 128 : (jt + 1) * 128],
                                    rhs=q_sb[rg : rg + 64, h, io + co : io + co + 512],
                                    start=True,
                                    stop=True,
                                    tile_position=(rg, 0),
                                )
                            es = exp_pool.tile([128, IH], BF16, name=f"es_{jt}", tag="es")
                            es_tiles[jt] = es
                            if jt in DVE_JTS:
                                nc.vector.tensor_scalar(
                                    out=es[:].bitcast(I16),
                                    in0=strip[:],
                                    scalar1=SCH_MUL,
                                    scalar2=SCH_ADD,
                                    op0=mybir.AluOpType.mult,
                                    op1=mybir.AluOpType.add,
                                )
                            else:
                                nc.scalar.activation(out=es[:], in_=strip[:], func=EXP)
                            # software-pipelined: AV lags sim by two j-tiles
                            # so its exp is long done when the PE reaches it
                            if jt >= 2:
                                emit_av(av, h, jt - 2)
                        emit_av(av, h, NJT - 2)
                        emit_av(av, h, NJT - 1)
                        # normalize: rows 64..127 of av already hold the
                        # normalizer replicated; shift it down via DMA, then
                        # one DVE divide finishes the softmax
                        ncp = norm_pool.tile([128, IH], F32, tag="ncp")
                        if h % 2 == 0:
                            nc.scalar.copy(out=ncp[DH:128, :], in_=av[DH:128, :])
                        else:
                            nc.vector.tensor_copy(out=ncp[DH:128, :], in_=av[DH:128, :])
                        nsh = norm_pool.tile([DH, IH], F32, tag="nsh")
                        nc.sync.dma_start(out=nsh[:], in_=ncp[DH:128, :])
                        # custom-DVE ops require base partition 0
                        nrm = norm_pool.tile([DH, IH], F32, tag="nrm")
                        nc.vector.reciprocal_approx_fast(out=nrm[:], in_=nsh[:])
                        if h % 2 == 0:
                            nc.vector.tensor_mul(
                                out=attout_sb[h // 2][0:DH, io : io + IH],
                                in0=av[0:DH, :],
                                in1=nrm[:],
                            )
                        else:
                            # odd heads land on partitions 64..127: bounce
                            atmp = norm_pool.tile([DH, IH], BF16, tag="atmp")
                            nc.vector.tensor_mul(
                                out=atmp[:], in0=av[0:DH, :], in1=nrm[:]
                            )
                            nc.sync.dma_start(
                                out=attout_sb[h // 2][DH:128, io : io + IH],
                                in_=atmp[:],
                            )

            # ---- phase 4: output projection + bias ----
            outr = out_d[:].rearrange("(ct p) w -> p ct w", p=128)
            with tc.tile_pool(name="proj_ps", bufs=8, space="PSUM") as proj_ps:
                for ot in range(NCT):
                    for wh in range(4):
                        wo = wh * 512
                        ps = proj_ps.tile([128, 512], F32, name=f"pj_{ot}_{wh}", tag="pj")
                        for kc in range(4):
                            nc.tensor.matmul(
                                ps[:],
                                lhsT=wo_sb[:, kc, ot * 128 : (ot + 1) * 128],
                                rhs=attout_sb[kc][:, wo : wo + 512],
                                start=(kc == 0),
                                stop=(kc == 3),
                            )
                        nc.vector.tensor_scalar_add(
                            out=out_sb[:, ot, wo : wo + 512],
                            in0=ps[:],
                            scalar1=bias_sb[:, ot : ot + 1],
                        )
                        nc.sync.dma_start(
                            out=outr[:, ot, wo : wo + 512],
                            in_=out_sb[:, ot, wo : wo + 512],
                        )

    nc.compile()
    return nc


_NC_CACHE = None


def _get_nc():
    global _NC_CACHE
    if _NC_CACHE is None:
        _NC_CACHE = build_kernel()
    return _NC_CACHE


def make_in_maps(x, w_qkv, w_out, b_out):
    bf16 = ml_dtypes.bfloat16
    wq = np.array(w_qkv, dtype=np.float32, copy=True)
    wq[:HID] *= SCALE  # fold attention scale into the q projection
    wqkvT = np.ascontiguousarray(wq.T).astype(bf16)  # [256, 1536]
    woutT = np.ascontiguousarray(
        w_out.T.reshape(4, 128, DIM).transpose(1, 0, 2)
    ).astype(bf16)  # [128, 4, 256]
    bias = np.ascontiguousarray(
        b_out.astype(np.float32).reshape(DIM // 128, 128).T
    )  # [128, 2]
    in_maps = []
    for i in range(N_CORES):
        in_maps.append(
            {
                "x": x[i].astype(bf16),
                "wqkvT": wqkvT,
                "woutT": woutT,
                "bias": bias,
            }
        )
    return in_maps


def kernel(x, w_qkv, w_out, b_out, _trace=False):
    nc = _get_nc()
    in_maps = make_in_maps(x, w_qkv, w_out, b_out)
    res = run_bass_kernel_spmd(
        nc,
        in_maps,
        core_ids=list(range(N_CORES)),
        trace=_trace,
        trace_cores=list(range(N_CORES)) if _trace else None,
    )
    out = np.stack([res.results[i]["out"] for i in range(N_CORES)], axis=0)
    if _trace:
        kernel.last_exec_time_ns = res.exec_time_ns
        kernel.last_results = res
    return out


# revision 16
# speedup vs baseline: 1.5289x; 1.0276x over previous
"""Trainium2 Bass kernel for nn_Attention_15539191677265.

Single-head-dim attention block:
    qkv = w_qkv @ x ; per-head scaled dot-product attention over w=2048;
    out = w_out @ attn_out + b_out

Sharding: pure data-parallel over batch (b=8 -> 8 NeuronCores, one batch
element per core). Weights are replicated. No collectives.

Per-core algorithm (transposed-softmax, bf16 matmuls):
  1. q,k projections, then DMA-duplicate each head's 64 d-rows into both
     PE row halves so consecutive sim matmuls alternate row groups.
  2. vT = x.T @ wvT with a ones column (AV psum row 64 accumulates the
     softmax normalizer for free).
  3. per (head, i-half 1024, j-tile): sim^T strip as two full-width K=64
     matmuls on alternating PE row-halves (tile_position (0,0)/(64,0)) --
     disjoint rows + different psum banks stream concurrently; exp as ONE
     [128,1024] op, split across engines: most j-tiles on ScalarE (exact
     exp), 6/16 on VectorE via the Schraudolph bf16 bit-trick
     (x*128/ln2 + magic -> int16 bits == bf16 exp approximation);
     AV accumulates [65, 1024] over the 16 j-tiles.
  4. normalize: reciprocal of the ones-row (DVE), broadcast (GpSimd),
     multiply (GpSimd), odd heads bounce partitions via DMA.
  5. proj: out = woutT.T @ attn_out (K=128 head-pair chunks) + bias.

No max subtraction before exp: scores are ~N(0,1) so exp cannot
overflow in bf16/fp32.
"""

import sys

if "/opt/trn_rl_repo" not in sys.path:
    sys.path.insert(0, "/opt/trn_rl_repo")

import numpy as np
import ml_dtypes

import concourse.bass as bass
import concourse.mybir as mybir
import concourse.tile as tile
from concourse import bacc
from concourse.bass_utils import run_bass_kernel_spmd

BF16 = mybir.dt.bfloat16
F32 = mybir.dt.float32
I16 = mybir.dt.int16
EXP = mybir.ActivationFunctionType.Exp

B, DIM, W = 8, 256, 2048
HEADS, DH = 8, 64
HID = HEADS * DH  # 512
SCALE = DH ** (-0.5)
N_CORES = 8

NJT = W // 128  # 16 j-tiles per head
NCT = DIM // 128  # 2 contraction chunks over channels
IH = 1024  # i-half width
NIH = W // IH

# j-tiles whose exp runs on VectorE via Schraudolph (rest: exact on ScalarE)
DVE_JTS = frozenset({1, 3, 5, 8, 10, 12, 14})
SCH_MUL = 128.0 / float(np.log(2.0))  # 184.6645
SCH_ADD = 127.0 * 128.0 - 7.4  # PWL-centering constant (round-to-nearest)


def build_kernel():
    nc = bacc.Bacc(None, target_bir_lowering=False)

    x_d = nc.dram_tensor("x", [DIM, W], BF16, kind="ExternalInput")
    wqkvT_d = nc.dram_tensor("wqkvT", [DIM, 3 * HID], BF16, kind="ExternalInput")
    woutT_d = nc.dram_tensor("woutT", [128, 4, DIM], BF16, kind="ExternalInput")
    bias_d = nc.dram_tensor("bias", [128, DIM // 128], F32, kind="ExternalInput")
    out_d = nc.dram_tensor("out", [DIM, W], F32, kind="ExternalOutput")

    with tile.TileContext(nc) as tc:
        with tc.tile_pool(name="pers", bufs=1) as pers:
            x_sb = pers.tile([128, NCT, W], BF16)
            wq_sb = pers.tile([128, NCT, 3 * HID], BF16)
            wo_sb = pers.tile([128, 4, DIM], BF16)
            bias_sb = pers.tile([128, DIM // 128], F32)
            # [128 = head d-rows duplicated in both halves, head, w]
            q_sb = pers.tile([128, HEADS, W], BF16)
            k_sb = pers.tile([128, HEADS, W], BF16)
            vt_sb = pers.tile([128, NJT, HEADS, 128], BF16)
            attout_sb = [
                pers.tile([128, W], BF16, name=f"attout_{kc}", tag=f"attout{kc}")
                for kc in range(4)
            ]
            out_sb = pers.tile([128, NCT, W], F32)

            xr = x_d[:].rearrange("(ct p) w -> p ct w", p=128)
            for ct in range(NCT):
                for wh in range(4):
                    nc.sync.dma_start(
                        out=x_sb[:, ct, wh * 512 : (wh + 1) * 512],
                        in_=xr[:, ct, wh * 512 : (wh + 1) * 512],
                    )
            wqr = wqkvT_d[:].rearrange("(ct p) o -> p ct o", p=128)
            for ct in range(NCT):
                for sec in range(3):
                    nc.sync.dma_start(
                        out=wq_sb[:, ct, sec * HID : (sec + 1) * HID],
                        in_=wqr[:, ct, sec * HID : (sec + 1) * HID],
                    )
            nc.sync.dma_start(out=wo_sb[:], in_=woutT_d[:])
            nc.sync.dma_start(out=bias_sb[:], in_=bias_d[:])

            # vT cols 64..127: ALL ones -> AV rows 64..127 hold the softmax
            # normalizer replicated across 64 partitions (pre-broadcast)
            nc.vector.memset(vt_sb[:, :, :, DH:128], 1.0)
            # warm the ACT exp table while qkv matmuls run
            warm = pers.tile([1, 1], F32)
            nc.vector.memset(warm[:], 0.0)
            nc.scalar.activation(out=warm[:], in_=warm[:], func=EXP)

            # ---- phase 1: v first (attention h0 needs all of vT), then q,k
            with tc.tile_pool(name="qkv_ps", bufs=4, space="PSUM") as qkv_ps:
                for jt in range(NJT):
                    ps = qkv_ps.tile([128, HID], F32, name=f"vt_{jt}", tag="vt")
                    for ct in range(NCT):
                        nc.tensor.matmul(
                            ps[:],
                            lhsT=x_sb[:, ct, jt * 128 : (jt + 1) * 128],
                            rhs=wq_sb[:, ct, 2 * HID : 3 * HID],
                            start=(ct == 0),
                            stop=(ct == NCT - 1),
                        )
                    if jt % 2 == 0:
                        nc.vector.tensor_copy(
                            out=vt_sb[:, jt, :, 0:DH],
                            in_=ps[:].rearrange("p (h d) -> p h d", h=HEADS),
                        )
                    else:
                        nc.scalar.copy(
                            out=vt_sb[:, jt, :, 0:DH],
                            in_=ps[:].rearrange("p (h d) -> p h d", h=HEADS),
                        )

                # q,k per o-tile (2 heads each); evac halves to their head
                # slots, then DMA duplicates into the other partition half
                for ot in range(4):
                    for dst, base in ((q_sb, 0), (k_sb, HID)):
                        for ph in range(4):
                            po = ph * 512
                            ps = qkv_ps.tile(
                                [128, 512], F32, name=f"qk_{ot}_{base}_{ph}", tag="qk"
                            )
                            for ct in range(NCT):
                                nc.tensor.matmul(
                                    ps[:],
                                    lhsT=wq_sb[:, ct, base + ot * 128 : base + (ot + 1) * 128],
                                    rhs=x_sb[:, ct, po : po + 512],
                                    start=(ct == 0),
                                    stop=(ct == NCT - 1),
                                )
                            if ph % 2 == 0:
                                nc.vector.tensor_copy(
                                    out=dst[0:64, 2 * ot, po : po + 512],
                                    in_=ps[0:64, :],
                                )
                                nc.scalar.copy(
                                    out=dst[64:128, 2 * ot + 1, po : po + 512],
                                    in_=ps[64:128, :],
                                )
                            else:
                                nc.scalar.copy(
                                    out=dst[0:64, 2 * ot, po : po + 512],
                                    in_=ps[0:64, :],
                                )
                                nc.vector.tensor_copy(
                                    out=dst[64:128, 2 * ot + 1, po : po + 512],
                                    in_=ps[64:128, :],
                                )
                        # partition-duplicating DMAs (overlap with matmuls)
                        nc.sync.dma_start(
                            out=dst[64:128, 2 * ot, :], in_=dst[0:64, 2 * ot, :]
                        )
                        nc.sync.dma_start(
                            out=dst[0:64, 2 * ot + 1, :], in_=dst[64:128, 2 * ot + 1, :]
                        )

            # ---- phase 3: attention ----
            with (
                tc.tile_pool(name="strip_ps", bufs=2, space="PSUM") as strip_ps,
                tc.tile_pool(name="av_ps", bufs=2, space="PSUM") as av_ps,
                tc.tile_pool(name="exp_sb", bufs=4) as exp_pool,
                tc.tile_pool(name="norm_sb", bufs=2) as norm_pool,
            ):
                def emit_av(av, h, jt):
                    es = es_tiles[jt]
                    for c in range(IH // 512):
                        co = c * 512
                        nc.tensor.matmul(
                            av[:, co : co + 512],
                            lhsT=vt_sb[:, jt, h, :],
                            rhs=es[:, co : co + 512],
                            start=(jt == 0),
                            stop=(jt == NJT - 1),
                        )

                for h in range(HEADS):
                    for ih in range(NIH):
                        io = ih * IH
                        av = av_ps.tile([128, IH], F32, name=f"av_{h}_{ih}", tag="av")
                        es_tiles = {}
                        for jt in range(NJT):
                            strip = strip_ps.tile(
                                [128, IH], F32, name=f"st_{h}_{ih}_{jt}", tag="st"
                            )
                            for c in range(IH // 512):
                                co = c * 512
                                # full-width K=64 matmuls on alternating PE
                                # row-halves: adjacent ops use disjoint rows
                                # and different psum banks -> concurrent
                                rg = 64 * (c % 2)
                                nc.tensor.matmul(
                                    strip[:, co : co + 512],
                                    lhsT=k_sb[rg : rg + 64, h, jt * 128 : (jt + 1) * 128],
                                    rhs=q_sb[rg : rg + 64, h, io + co : io + co + 512],
                                    start=True,
                                    stop=True,
                                    tile_position=(rg, 0),
                                )
                            es = exp_pool.tile([128, IH], BF16, name=f"es_{jt}", tag="es")
                            es_tiles[jt] = es
                            if jt in DVE_JTS:
                                nc.vector.tensor_scalar(
                                    out=es[:].bitcast(I16),
                                    in0=strip[:],
                                    scalar1=SCH_MUL,
                                    scalar2=SCH_ADD,
                                    op0=mybir.AluOpType.mult,
                                    op1=mybir.AluOpType.add,
                                )
                            else:
                                nc.scalar.activation(out=es[:], in_=strip[:], func=EXP)
                            # software-pipelined: AV lags sim by two j-tiles
                            # so its exp is long done when the PE reaches it
                            if jt >= 2:
                                emit_av(av, h, jt - 2)
                        emit_av(av, h, NJT - 2)
                        emit_av(av, h, NJT - 1)
                        # normalize: rows 64..127 of av already hold the
                        # normalizer replicated; shift it down via DMA, then
                        # one DVE divide finishes the softmax
                        ncp = norm_pool.tile([128, IH], F32, tag="ncp")
                        if h % 2 == 0:
                            nc.scalar.copy(out=ncp[:], in_=av[:])
                        else:
                            nc.vector.tensor_copy(out=ncp[:], in_=av[:])
                        nsh = norm_pool.tile([DH, IH], F32, tag="nsh")
                        nc.sync.dma_start(out=nsh[:], in_=ncp[DH:128, :])
                        # custom-DVE recip requires base partition 0
                        nrm = norm_pool.tile([DH, IH], F32, tag="nrm")
                        nc.vector.reciprocal_approx_fast(out=nrm[:], in_=nsh[:])
                        # SBUF-only multiply on the otherwise-idle GpSimd
                        if h % 2 == 0:
                            nc.gpsimd.tensor_mul(
                                out=attout_sb[h // 2][0:DH, io : io + IH],
                                in0=ncp[0:DH, :],
                                in1=nrm[:],
                            )
                        else:
                            # odd heads land on partitions 64..127: bounce
                            atmp = norm_pool.tile([DH, IH], BF16, tag="atmp")
                            nc.gpsimd.tensor_mul(
                                out=atmp[:], in0=ncp[0:DH, :], in1=nrm[:]
                            )
                            nc.sync.dma_start(
                                out=attout_sb[h // 2][DH:128, io : io + IH],
                                in_=atmp[:],
                            )

            # ---- phase 4: output projection + bias ----
            outr = out_d[:].rearrange("(ct p) w -> p ct w", p=128)
            with tc.tile_pool(name="proj_ps", bufs=8, space="PSUM") as proj_ps:
                for ot in range(NCT):
                    for wh in range(4):
                        wo = wh * 512
                        ps = proj_ps.tile([128, 512], F32, name=f"pj_{ot}_{wh}", tag="pj")
                        for kc in range(4):
                            nc.tensor.matmul(
                                ps[:],
                                lhsT=wo_sb[:, kc, ot * 128 : (ot + 1) * 128],
                                rhs=attout_sb[kc][:, wo : wo + 512],
                                start=(kc == 0),
                                stop=(kc == 3),
                            )
                        nc.vector.tensor_scalar_add(
                            out=out_sb[:, ot, wo : wo + 512],
                            in0=ps[:],
                            scalar1=bias_sb[:, ot : ot + 1],
                        )
                        nc.sync.dma_start(
                            out=outr[:, ot, wo : wo + 512],
                            in_=out_sb[:, ot, wo : wo + 512],
                        )

    nc.compile()
    return nc


_NC_CACHE = None


def _get_nc():
    global _NC_CACHE
    if _NC_CACHE is None:
        _NC_CACHE = build_kernel()
    return _NC_CACHE


def make_in_maps(x, w_qkv, w_out, b_out):
    bf16 = ml_dtypes.bfloat16
    wq = np.array(w_qkv, dtype=np.float32, copy=True)
    wq[:HID] *= SCALE  # fold attention scale into the q projection
    wqkvT = np.ascontiguousarray(wq.T).astype(bf16)  # [256, 1536]
    woutT = np.ascontiguousarray(
        w_out.T.reshape(4, 128, DIM).transpose(1, 0, 2)
    ).astype(bf16)  # [128, 4, 256]
    bias = np.ascontiguousarray(
        b_out.astype(np.float32).reshape(DIM // 128, 128).T
    )  # [128, 2]
    in_maps = []
    for i in range(N_CORES):
        in_maps.append(
            {
                "x": x[i].astype(bf16),
                "wqkvT": wqkvT,
                "woutT": woutT,
                "bias": bias,
            }
        )
    return in_maps


def kernel(x, w_qkv, w_out, b_out, _trace=False):
    nc = _get_nc()
    in_maps = make_in_maps(x, w_qkv, w_out, b_out)
    res = run_bass_kernel_spmd(
        nc,
        in_maps,
        core_ids=list(range(N_CORES)),
        trace=_trace,
        trace_cores=list(range(N_CORES)) if _trace else None,
    )
    out = np.stack([res.results[i]["out"] for i in range(N_CORES)], axis=0)
    if _trace:
        kernel.last_exec_time_ns = res.exec_time_ns
        kernel.last_results = res
    return out
